# revision 1
# baseline (speedup 1.0000x reference)
"""GPU-preprocessor kernel for Trainium2 (Bass/Tile), 8-core data parallel.

Pipeline per image (NHWC f32 [1280, 960, 3] -> NCHW f32 [3, 640, 640]):
  1. bilinear resize 1280x960 -> 640x640, half-pixel centers, no antialias
     - H: exact 2x downscale -> out_row i = 0.5*(row 2i + row 2i+1)
     - W: 1.5x downscale, period 3 px -> 2 px:
         out j=2k   = 0.75*px[3k]   + 0.25*px[3k+1]
         out j=2k+1 = 0.25*px[3k+1] + 0.75*px[3k+2]
  2. x/255, (x-mean)/std folded into one affine per channel applied last:
     out = R * s_c + b_c with s_c = 1/(255*std_c), b_c = -mean_c/std_c
     (valid because resize weights sum to 1 at every stage).

Engine split per 128-row tile:
  - DMA (HWDGE/sync): one contiguous load [128, 5760] (row pairs), store [128, 3, 640]
  - DVE: v = e + o (2880); t0 = gather_l(v) * WA; t1 = gather_r(v) * WB (planar writes)
  - GPSIMD: s = t0 + t1 (contiguous planar)
  - ACT: out_c = s_c * scale_c + bias_c (per-partition scalar APs from mean/std)
"""

import numpy as np
from contextlib import ExitStack

import concourse.mybir as mybir
from concourse import bass
from concourse import tile
from concourse.bass_utils import run_bass_kernel_spmd

F32 = mybir.dt.float32

N_CORES = 8
B_FULL = 16
H_IN, W_IN, C = 1280, 960, 3
H_OUT, W_OUT = 640, 640
PER_B = B_FULL // N_CORES          # 2 images per core
TILE_P = 128                       # output rows per tile
N_TILES = H_OUT // TILE_P          # 5 tiles per image
FREE_IN = W_IN * C                 # 2880 floats per input row
FREE_PAIR = 2 * FREE_IN            # 5760 floats per row-pair
FREE_OUT = W_OUT * C               # 1920 floats per output row

_BUILt_CACHE = {}


def _build_nc():
    nc = bass.Bass()
    img = nc.declare_dram_parameter("images", [PER_B, H_IN, W_IN, C], F32, isOutput=False)
    sb = nc.declare_dram_parameter("sb", [TILE_P, 8], F32, isOutput=False)
    out = nc.declare_dram_parameter("out", [PER_B, C, H_OUT, W_OUT], F32, isOutput=True)

    with tile.TileContext(nc) as tc, ExitStack() as ctx:
        const_pool = ctx.enter_context(tc.tile_pool(name="const", bufs=1))
        in_pool = ctx.enter_context(tc.tile_pool(name="inp", bufs=3))
        t_pool = ctx.enter_context(tc.tile_pool(name="t", bufs=4))
        s_pool = ctx.enter_context(tc.tile_pool(name="s", bufs=4))
        o_pool = ctx.enter_context(tc.tile_pool(name="o", bufs=4))

        # Constants: interleaved weight tiles [128, 1920] viewed as (k=320, six).
        # Out float m = 6k+i: i in 0..2 -> even out px (left w 0.375), i in 3..5
        # -> odd out px (left w 0.125). Right weights swapped. (0.5 vertical
        # average folded in.)
        wa = const_pool.tile([TILE_P, FREE_OUT], F32, tag="wa")
        wb = const_pool.tile([TILE_P, FREE_OUT], F32, tag="wb")
        wa_v = wa[:].rearrange("p (k six) -> p k six", six=6)
        wb_v = wb[:].rearrange("p (k six) -> p k six", six=6)
        nc.vector.memset(wa_v[:, :, 0:3], 0.375)
        nc.vector.memset(wa_v[:, :, 3:6], 0.125)
        nc.vector.memset(wb_v[:, :, 0:3], 0.125)
        nc.vector.memset(wb_v[:, :, 3:6], 0.375)

        sbt_raw = const_pool.tile([TILE_P, 8], F32, tag="sbt_raw")
        nc.sync.dma_start(sbt_raw[:], sb[:])
        # DVE-owned copy so downstream tensor_scalar ops don't need a DMA wait
        sbt = const_pool.tile([TILE_P, 8], F32, tag="sbt")
        nc.vector.tensor_copy(sbt[:], sbt_raw[:])

        for b in range(PER_B):
            # [640 row-pairs, 5760 floats] contiguous per pair
            src_pairs = img[b].rearrange("(pair two) w c -> pair (two w c)", two=2)
            dst_rows = out[b].rearrange("c h w -> h c w")  # [640, 3, 640]
            for ti in range(N_TILES):
                i0 = ti * TILE_P

                tin = in_pool.tile([TILE_P, FREE_PAIR], F32, tag="tin")
                # SWDGE loads: keeps load issues off the SP ring, where store
                # waits (on ACT output) would head-of-line-block them.
                nc.gpsimd.dma_start(tin[:], src_pairs[i0:i0 + TILE_P, :])

                e = tin[:, 0:FREE_IN]
                o = tin[:, FREE_IN:FREE_PAIR]
                # vertical add in-place into the e-half (frees a whole pool;
                # DVE streams element reads ahead of writes, same-index safe)
                v = e
                nc.vector.tensor_add(v, e, o)

                # windows of v: [p, k, 9] -> left run 0:6, right run 3:9
                # (contiguous innermost runs; out float m = 6k+i).
                v9 = v.rearrange("p (k nine) -> p k nine", nine=9)
                v_l = v9[:, :, 0:6]
                v_r = v9[:, :, 3:9]

                t0 = t_pool.tile([TILE_P, FREE_OUT], F32, tag="t0")
                t1 = t_pool.tile([TILE_P, FREE_OUT], F32, tag="t1")
                t0_v = t0[:].rearrange("p (k six) -> p k six", six=6)
                t1_v = t1[:].rearrange("p (k six) -> p k six", six=6)
                nc.vector.tensor_mul(t0_v, v_l, wa_v)
                nc.vector.tensor_mul(t1_v, v_r, wb_v)

                s = s_pool.tile([TILE_P, FREE_OUT], F32, tag="s")
                nc.gpsimd.tensor_add(s[:], t0[:], t1[:])

                ot = o_pool.tile([TILE_P, FREE_OUT], F32, tag="ot")
                # s is px-interleaved (j, c); final affine deinterleaves to
                # planar (c, j) on the otherwise-idle Scalar engine.
                s_v = s[:].rearrange("p (j c) -> p c j", c=C)
                o3 = ot[:].rearrange("p (c j) -> p c j", c=C)
                for c in range(C):
                    nc.scalar.activation(
                        o3[:, c], s_v[:, c],
                        mybir.ActivationFunctionType.Identity,
                        bias=sbt[:, 4 + c:5 + c],
                        scale=sbt[:, c:c + 1],
                    )

                nc.sync.dma_start(dst_rows[i0:i0 + TILE_P, :, :], o3)

    return nc


def _split_multi_waits(nc):
    """walrus codegen accepts at most one semaphore wait per instruction;
    this Tile version can leave several in sync_info.on_wait. Move the
    extras onto same-engine InstNoOp carriers inserted just before."""
    n_split = 0
    for bb in nc.main_func.blocks:
        new_insts = []
        for ins in bb.instructions:
            si = ins.sync_info
            if si is not None and si.on_wait is not None and len(si.on_wait) > 1:
                waits = list(si.on_wait)
                for w in waits[:-1]:
                    nop = mybir.InstNoOp(
                        name=nc.get_next_instruction_name(),
                        engine=ins.engine,
                        ins=[],
                        outs=[],
                        sync_info=mybir.SyncInfo(on_wait=[w], on_update=[]),
                    )
                    new_insts.append(nop)
                ins.sync_info = mybir.SyncInfo(
                    on_wait=[waits[-1]], on_update=list(si.on_update or [])
                )
                n_split += 1
            new_insts.append(ins)
        bb.instructions[:] = new_insts
    return n_split


def _get_nc():
    if "nc" not in _BUILt_CACHE:
        nc = _build_nc()
        _split_multi_waits(nc)
        _BUILt_CACHE["nc"] = nc
    return _BUILt_CACHE["nc"]


def run(images, mean, std, trace=False, **spmd_kwargs):
    images = np.ascontiguousarray(np.asarray(images, dtype=np.float32))
    mean = np.asarray(mean, dtype=np.float32).reshape(-1)
    std = np.asarray(std, dtype=np.float32).reshape(-1)
    assert images.shape == (B_FULL, H_IN, W_IN, C), images.shape

    scale = 1.0 / (255.0 * std.astype(np.float64))
    bias = -(mean.astype(np.float64) / std.astype(np.float64))
    sbarr = np.zeros((TILE_P, 8), dtype=np.float32)
    sbarr[:, 0:3] = scale.astype(np.float32)
    sbarr[:, 4:7] = bias.astype(np.float32)

    nc = _get_nc()
    in_maps = [
        {"images": np.ascontiguousarray(images[i * PER_B:(i + 1) * PER_B]), "sb": sbarr}
        for i in range(N_CORES)
    ]
    res = run_bass_kernel_spmd(nc, in_maps, list(range(N_CORES)), trace=trace, **spmd_kwargs)
    outs = np.concatenate([r["out"] for r in res.results], axis=0)
    return outs, res


def kernel(**inputs):
    outs, _ = run(inputs["images"], inputs["mean"], inputs["std"], trace=False)
    return outs



# revision 6
# speedup vs baseline: 1.4890x; 1.4890x over previous
"""GPU-preprocessor kernel for Trainium2 (Bass/Tile), 8-core data parallel.

Pipeline per image (NHWC f32 [1280, 960, 3] -> NCHW f32 [3, 640, 640]):
  1. bilinear resize 1280x960 -> 640x640, half-pixel centers, no antialias
     - H: exact 2x downscale -> out_row i = 0.5*(row 2i + row 2i+1)
     - W: 1.5x downscale, period 3 px -> 2 px:
         out j=2k   = 0.75*px[3k]   + 0.25*px[3k+1]
         out j=2k+1 = 0.25*px[3k+1] + 0.75*px[3k+2]
  2. x/255, (x-mean)/std folded into one affine per channel applied last.

Per 128-row tile (v = e + o is the vertical pair-sum, so out px pre-affine
is 0.125*(3*v_near + v_mid)):
  - DMA (SWDGE via gpsimd): one contiguous load [128, 5760] (row pairs)
  - GPSIMD+DVE: v = e + o (2880, column-split across both engines)
  - DVE: scalar_tensor_tensor t[even] = (v_l * 3 + v_m) and
    t[odd] = (v_r * 3 + v_m), written planar (c-major)
    (TensorScalarPtr is DVE-only on core v3; GPSIMD rejects it)
  - ACT: per channel plane, out_c = t_c * (0.125*s_c) + b_c with
    s_c = 1/(255*std_c), b_c = -mean_c/std_c (contiguous reads)
  - DMA (HWDGE via sync): store [128, 3, 640]
"""

import numpy as np
from contextlib import ExitStack

import concourse.mybir as mybir
from concourse import bass
from concourse import tile
from concourse.bass_utils import run_bass_kernel_spmd

F32 = mybir.dt.float32

N_CORES = 8
B_FULL = 16
H_IN, W_IN, C = 1280, 960, 3
H_OUT, W_OUT = 640, 640
PER_B = B_FULL // N_CORES          # 2 images per core
TILE_P = 128                       # output rows per tile
N_TILES = H_OUT // TILE_P          # 5 tiles per image
FREE_IN = W_IN * C                 # 2880 floats per input row
FREE_PAIR = 2 * FREE_IN            # 5760 floats per row-pair
FREE_OUT = W_OUT * C               # 1920 floats per output row
V_SPLIT = 1350                     # GPSIMD's share of the vertical add

_BUILT_CACHE = {}


def _build_nc():
    nc = bass.Bass()
    img = nc.declare_dram_parameter("images", [PER_B, H_IN, W_IN, C], F32, isOutput=False)
    sb = nc.declare_dram_parameter("sb", [TILE_P, 8], F32, isOutput=False)
    out = nc.declare_dram_parameter("out", [PER_B, C, H_OUT, W_OUT], F32, isOutput=True)

    with tile.TileContext(nc) as tc, ExitStack() as ctx:
        const_pool = ctx.enter_context(tc.tile_pool(name="const", bufs=1))
        in_pool = ctx.enter_context(tc.tile_pool(name="inp", bufs=4))
        t_pool = ctx.enter_context(tc.tile_pool(name="t", bufs=3))
        o_pool = ctx.enter_context(tc.tile_pool(name="o", bufs=3))

        sbt_raw = const_pool.tile([TILE_P, 8], F32, tag="sbt_raw")
        nc.sync.dma_start(sbt_raw[:], sb[:])
        # DVE-owned copy so downstream ACT ops don't need a DMA wait
        sbt = const_pool.tile([TILE_P, 8], F32, tag="sbt")
        nc.vector.tensor_copy(sbt[:], sbt_raw[:])

        for b in range(PER_B):
            # [640 row-pairs, 5760 floats] contiguous per pair
            src_pairs = img[b].rearrange("(pair two) w c -> pair (two w c)", two=2)
            dst_rows = out[b].rearrange("c h w -> h c w")  # [640, 3, 640]
            for ti in range(N_TILES):
                i0 = ti * TILE_P

                tin = in_pool.tile([TILE_P, FREE_PAIR], F32, tag="tin")
                # SWDGE loads: keeps load issues off the SP ring, where store
                # waits (on ACT output) would head-of-line-block them.
                nc.gpsimd.dma_start(tin[:], src_pairs[i0:i0 + TILE_P, :])

                e = tin[:, 0:FREE_IN]
                o = tin[:, FREE_IN:FREE_PAIR]
                # vertical add in-place into the e-half (engines stream
                # element reads ahead of writes, same-index safe), split
                # DVE/GPSIMD to balance engine load
                v = e
                nc.gpsimd.tensor_add(v[:, 0:V_SPLIT], e[:, 0:V_SPLIT], o[:, 0:V_SPLIT])
                nc.vector.tensor_add(v[:, V_SPLIT:], e[:, V_SPLIT:], o[:, V_SPLIT:])

                # windows of v: [p, k, 9]; px 3k/3k+1/3k+2 are floats
                # 0:3 / 3:6 / 6:9 of each 9-group
                v9 = v.rearrange("p (k nine) -> p k nine", nine=9)
                v_l = v9[:, :, 0:3]
                v_m = v9[:, :, 3:6]
                v_r = v9[:, :, 6:9]

                # pre-affine output, planar (c-major), px-parity interleave:
                # flat index = c*640 + 2k + parity
                t = t_pool.tile([TILE_P, FREE_OUT], F32, tag="t")
                tv = t[:].rearrange("p (c k two) -> p k c two", c=C, two=2)
                nc.vector.scalar_tensor_tensor(
                    tv[:, :, :, 0], v_l, 3.0, v_m,
                    mybir.AluOpType.mult, mybir.AluOpType.add)
                nc.vector.scalar_tensor_tensor(
                    tv[:, :, :, 1], v_r, 3.0, v_m,
                    mybir.AluOpType.mult, mybir.AluOpType.add)

                # per-channel affine on contiguous planes (scale = 0.125*s_c)
                ot = o_pool.tile([TILE_P, FREE_OUT], F32, tag="ot")
                t3 = t[:].rearrange("p (c j) -> p c j", c=C)
                o3 = ot[:].rearrange("p (c j) -> p c j", c=C)
                for c in range(C):
                    nc.scalar.activation(
                        o3[:, c], t3[:, c],
                        mybir.ActivationFunctionType.Identity,
                        bias=sbt[:, 4 + c:5 + c],
                        scale=sbt[:, c:c + 1],
                    )

                nc.sync.dma_start(dst_rows[i0:i0 + TILE_P, :, :], o3)

    return nc


def _split_multi_waits(nc):
    """walrus codegen accepts at most one semaphore wait per instruction;
    this Tile version can leave several in sync_info.on_wait. Move the
    extras onto same-engine InstNoOp carriers inserted just before."""
    n_split = 0
    for bb in nc.main_func.blocks:
        new_insts = []
        for ins in bb.instructions:
            si = ins.sync_info
            if si is not None and si.on_wait is not None and len(si.on_wait) > 1:
                waits = list(si.on_wait)
                for w in waits[:-1]:
                    nop = mybir.InstNoOp(
                        name=nc.get_next_instruction_name(),
                        engine=ins.engine,
                        ins=[],
                        outs=[],
                        sync_info=mybir.SyncInfo(on_wait=[w], on_update=[]),
                    )
                    new_insts.append(nop)
                ins.sync_info = mybir.SyncInfo(
                    on_wait=[waits[-1]], on_update=list(si.on_update or [])
                )
                n_split += 1
            new_insts.append(ins)
        bb.instructions[:] = new_insts
    return n_split


def _get_nc():
    if "nc" not in _BUILT_CACHE:
        nc = _build_nc()
        _split_multi_waits(nc)
        _BUILT_CACHE["nc"] = nc
    return _BUILT_CACHE["nc"]


def run(images, mean, std, trace=False, **spmd_kwargs):
    images = np.ascontiguousarray(np.asarray(images, dtype=np.float32))
    mean = np.asarray(mean, dtype=np.float32).reshape(-1)
    std = np.asarray(std, dtype=np.float32).reshape(-1)
    assert images.shape == (B_FULL, H_IN, W_IN, C), images.shape

    # ACT input is 8x the resized value (3+1 weights on v = 2x vertical sum)
    scale = 0.125 / (255.0 * std.astype(np.float64))
    bias = -(mean.astype(np.float64) / std.astype(np.float64))
    sbarr = np.zeros((TILE_P, 8), dtype=np.float32)
    sbarr[:, 0:3] = scale.astype(np.float32)
    sbarr[:, 4:7] = bias.astype(np.float32)

    nc = _get_nc()
    in_maps = [
        {"images": np.ascontiguousarray(images[i * PER_B:(i + 1) * PER_B]), "sb": sbarr}
        for i in range(N_CORES)
    ]
    res = run_bass_kernel_spmd(nc, in_maps, list(range(N_CORES)), trace=trace, **spmd_kwargs)
    outs = np.concatenate([r["out"] for r in res.results], axis=0)
    return outs, res


def kernel(**inputs):
    outs, _ = run(inputs["images"], inputs["mean"], inputs["std"], trace=False)
    return outs
